# revision 1
# baseline (speedup 1.0000x reference)
"""BitLinear (LayerNorm + absmax-quantize + binary-weight matmul) on 8 trn2 cores.

Sharding: data-parallel over tokens. Each core gets T/8 tokens of x and the
full weight matrix; LayerNorm+quantize are computed per-token on the owning
core, so nothing is replicated work-wise and no collectives are needed.

Per-core pipeline:
  phase 1 (per 128-token group): LN stats via bn_stats/bn_aggr, fused
    (x-mu)*rsqrt(var+eps)*(QB/gamma) via one ACT pass, clip+cast to bf16 via
    one DVE tensor_scalar, then XBAR dma-transpose to feature-major layout.
  phase 2: token-tile stationary / w moving matmul, PSUM-accumulated over the
    32 k-tiles, scaled by beta*gamma/QB on the PSUM->SBUF copy, written back
    in natural [token, n_out] layout. Two token-half passes so the PE can
    start on tokens 0..T/2 while LN of the second half is still running.
"""

import functools
import sys
from contextlib import ExitStack

sys.path.insert(0, "/opt/trn_rl_repo")

import ml_dtypes
import numpy as np

import concourse.bass as bass
import concourse.mybir as mybir
import concourse.tile as tile
from concourse import bacc
from concourse.bass_utils import run_bass_kernel_spmd

N_CORES = 8
P = 128
QB = 128.0
EP = 0.01
LN_EPS = 1e-5

F32 = mybir.dt.float32
BF16 = mybir.dt.bfloat16


FP8 = mybir.dt.float8e4


def build(T, D, NOUT, s, out_scale, with_ln_affine, n_passes=2, jn_block=512,
          repeat=1, emit_phase1=True, emit_phase2=True, w_dt=BF16,
          dve_copy=False):
    """Emit + compile the per-core program.

    T: tokens per core, D: n_in, NOUT: n_out. s = QB/gamma.
    with_ln_affine: apply ln_gamma/ln_beta tensors (skipped when they are
    the identity, which is what the reference's setup produces).
    """
    assert T % P == 0 and D % P == 0 and NOUT % jn_block == 0
    G = T // P          # token groups
    KT = D // P         # contraction tiles
    JN = NOUT // jn_block
    n_bn = (D + 511) // 512
    assert D % n_bn == 0
    bn_w = D // n_bn
    assert G % n_passes == 0
    g_per_pass = G // n_passes

    nc = bacc.Bacc("TRN2", target_bir_lowering=False, debug=False)
    x = nc.declare_dram_parameter("x", [T, D], F32, isOutput=False).ap()
    w = nc.declare_dram_parameter("w", [D, NOUT], w_dt, isOutput=False).ap()
    y = nc.declare_dram_parameter("y", [T, NOUT], F32, isOutput=True).ap()
    if with_ln_affine:
        ln_g = nc.declare_dram_parameter("ln_g", [D], F32, isOutput=False).ap()
        ln_bs = nc.declare_dram_parameter("ln_bs", [D], F32, isOutput=False).ap()

    clip_hi = float(np.float32(QB) - np.float32(EP))
    inv_s2 = float(1.0 / (np.float64(s) * np.float64(s)))
    eps_s2 = float(np.float64(LN_EPS) * inv_s2)

    with tile.TileContext(nc) as tc, ExitStack() as ctx:
        singles = ctx.enter_context(tc.tile_pool(name="singles", bufs=1))
        xin = ctx.enter_context(tc.tile_pool(name="xin", bufs=3))
        xqp = ctx.enter_context(tc.tile_pool(name="xqp", bufs=3))
        st = ctx.enter_context(tc.tile_pool(name="st", bufs=4))
        xqT_pool = ctx.enter_context(tc.tile_pool(name="xqT", bufs=G))
        wpool = ctx.enter_context(tc.tile_pool(name="wpool", bufs=8))
        ysb = ctx.enter_context(tc.tile_pool(name="ysb", bufs=8))
        psum = ctx.enter_context(tc.tile_pool(
            name="psum", bufs=max(1, 8 // max(1, jn_block // 512)),
            space="PSUM"))

        # eps tile holds eps/s^2 so that 1/sqrt(var/s^2 + eps/s^2) = s*rstd
        eps_t = singles.tile([P, 1], F32)
        nc.vector.memset(eps_t, eps_s2)

        if with_ln_affine:
            g_b = singles.tile([P, D], F32)
            bs_b = singles.tile([P, D], F32)
            for vec, dst in ((ln_g, g_b), (ln_bs, bs_b)):
                bcast = bass.AP(tensor=vec.tensor, offset=vec.offset,
                                ap=[[0, P]] + list(vec.ap))
                nc.sync.dma_start(out=dst, in_=bcast)

        def emit_phase1_group(g, xqT):
            if not emit_phase1:
                xqT_g = xqT_pool.tile([P, KT, P], BF16, tag="xqT", name="xqT_g")
                nc.gpsimd.memset(xqT_g, 0)
                xqT.append(xqT_g)
                return
            x_t = xin.tile([P, D], F32)
            nc.sync.dma_start(out=x_t, in_=x[g * P:(g + 1) * P, :])

            stats = st.tile([P, n_bn, 6], F32)
            xv = x_t.rearrange("p (n b) -> p n b", n=n_bn)
            for sg in range(n_bn):
                nc.vector.bn_stats(out=stats[:, sg, :], in_=xv[:, sg, :])
            mv = st.tile([P, 2], F32)
            nc.vector.bn_aggr(out=mv, in_=stats)

            # srstd = s / sqrt(var + eps) = 1 / sqrt(var/s^2 + eps/s^2)
            srstd = st.tile([P, 1], F32)
            nc.scalar.activation(out=srstd, in_=mv[:, 1:2],
                                 func=mybir.ActivationFunctionType.Sqrt,
                                 bias=eps_t, scale=inv_s2)
            nc.vector.reciprocal(out=srstd, in_=srstd)
            # b = -mu * srstd
            b_t = st.tile([P, 1], F32)
            nc.vector.tensor_scalar(b_t, mv[:, 0:1], srstd, -1.0,
                                    mybir.AluOpType.mult, mybir.AluOpType.mult)
            # x_t = x*srstd + b = (x - mu) * rstd * s
            nc.scalar.activation(out=x_t, in_=x_t,
                                 func=mybir.ActivationFunctionType.Identity,
                                 bias=b_t, scale=srstd)
            if with_ln_affine:
                nc.vector.tensor_tensor(x_t, x_t, g_b, mybir.AluOpType.mult)
                nc.vector.tensor_tensor(x_t, x_t, bs_b, mybir.AluOpType.add)
            xq = xqp.tile([P, D], BF16)
            nc.vector.tensor_scalar(xq, x_t, clip_hi, -clip_hi,
                                    mybir.AluOpType.min, mybir.AluOpType.max)
            xqT_g = xqT_pool.tile([P, KT, P], BF16, tag="xqT")
            nc.sync.dma_start_transpose(xqT_g, xq)
            xqT.append(xqT_g)

        NB = jn_block // 512  # matmuls (PSUM banks) per stationary load
        assert g_per_pass * NB <= 8, "PSUM banks exceeded"

        def emit_pass(p_i, xqT):
            # matmul pass: stationary = token tile, moving = w columns
            toks = range(p_i * g_per_pass, (p_i + 1) * g_per_pass)
            for jn in range(JN):
                ps = {t: psum.tile([P, NB, 512], F32, tag="ps",
                                   name=f"ps_{t}")
                      for t in toks}
                for kt in range(KT):
                    w_t = wpool.tile([P, jn_block], w_dt)
                    nc.sync.dma_start(
                        out=w_t,
                        in_=w[kt * P:(kt + 1) * P,
                              jn * jn_block:(jn + 1) * jn_block])
                    for t in toks:
                        for nb in range(NB):
                            nc.tensor.matmul(
                                ps[t][:, nb, :], xqT[t][:, kt, :],
                                w_t[:, nb * 512:(nb + 1) * 512],
                                start=(kt == 0), stop=(kt == KT - 1))
                for t in toks:
                    yo = ysb.tile([P, jn_block], F32)
                    if dve_copy:
                        nc.vector.tensor_scalar_mul(
                            yo, ps[t].rearrange("p a b -> p (a b)"), out_scale)
                    else:
                        nc.scalar.mul(out=yo,
                                      in_=ps[t].rearrange("p a b -> p (a b)"),
                                      mul=out_scale)
                    nc.sync.dma_start(
                        out=y[t * P:(t + 1) * P,
                              jn * jn_block:(jn + 1) * jn_block],
                        in_=yo)

        def emit_once():
            xqT = []
            # interleave: LN for each token-half right before its matmul
            # pass, so pass p's copies aren't queued behind half p+1's
            # elementwise work on the same engines.
            for g in range(g_per_pass):
                emit_phase1_group(g, xqT)
            for p_i in range(n_passes):
                if p_i + 1 < n_passes:
                    for g in range((p_i + 1) * g_per_pass,
                                   (p_i + 2) * g_per_pass):
                        emit_phase1_group(g, xqT)
                if emit_phase2:
                    emit_pass(p_i, xqT)
            if not emit_phase2:
                for g in range(G):
                    yo = ysb.tile([P, 8], F32, name="yo_dummy")
                    nc.vector.tensor_copy(yo, xqT[g][:, 0, 0:8])
                    nc.sync.dma_start(out=y[g * P:(g + 1) * P, 0:8], in_=yo)

        for _ in range(repeat):
            emit_once()

    nc.compile()
    return nc


# Best measured config (see work/ benchmarks): fp8 w halves weight DMA,
# jn_block amortizes one stationary (LDWEIGHTS) over jn_block/512 matmuls.
BEST = dict(jn_block=1024, n_passes=2, dve_copy=True)


@functools.lru_cache(maxsize=4)
def _built(T, D, NOUT, s, out_scale, with_ln_affine, w_is_fp8):
    return build(T, D, NOUT, s, out_scale, with_ln_affine,
                 w_dt=FP8 if w_is_fp8 else BF16, **BEST)


def kernel(x, w, ln_gamma, ln_beta, beta, gamma):
    B, S, D = x.shape
    NOUT = w.shape[1]
    T_full = B * S
    assert T_full % N_CORES == 0
    T = T_full // N_CORES

    gamma32 = np.float32(gamma)
    s = float(np.float32(QB) / gamma32)
    out_scale = float(np.float32(beta) * gamma32 / np.float32(QB))
    with_ln_affine = not (np.all(ln_gamma == 1.0) and np.all(ln_beta == 0.0))

    # w is +-1 in this problem, which fp8e4m3 represents exactly; fall back
    # to bf16 if some future w isn't exactly representable in fp8.
    fp8_np = mybir.dt.np(FP8)
    w_fp8 = np.asarray(w, dtype=np.float32).astype(fp8_np)
    w_is_fp8 = bool(np.array_equal(w_fp8.astype(np.float32),
                                   np.asarray(w, dtype=np.float32)))
    w_dev = w_fp8 if w_is_fp8 else np.asarray(w).astype(ml_dtypes.bfloat16)

    nc = _built(T, D, NOUT, s, out_scale, with_ln_affine, w_is_fp8)

    x_flat = np.ascontiguousarray(x.reshape(T_full, D), dtype=np.float32)
    in_maps = []
    for c in range(N_CORES):
        m = {"x": x_flat[c * T:(c + 1) * T], "w": w_dev}
        if with_ln_affine:
            m["ln_g"] = np.asarray(ln_gamma, dtype=np.float32)
            m["ln_bs"] = np.asarray(ln_beta, dtype=np.float32) * np.float32(s)
        in_maps.append(m)

    res = run_bass_kernel_spmd(nc, in_maps, list(range(N_CORES)))
    out = np.concatenate([res.results[c]["y"] for c in range(N_CORES)], axis=0)
    return out.reshape(B, S, NOUT).astype(np.float32)

